# revision 34
# baseline (speedup 1.0000x reference)
"""Trainium2 Bass kernel: grouped similarity-gating normalization (bf16 I/O).

Reference computation (per batch b, group g, cpg=64 channels, hw=784):
    means[c]  = mean_hw(x[c, :])
    s[hw]     = sum_c x[c, hw] * means[c]
    t         = (s - mean(s)) * rsqrt(var(s) + eps)
    gate      = sigmoid(t * weight[g] + bias[g])
    out[c,hw] = x[c, hw] * gate[hw]

Sharding: data-parallel over batch B=64 across 8 cores (8 batches/core).
Harness gate is rel_err < 2e-2; x is bf16 on the wire (halves HBM traffic
-> ~36us DMA roofline/core), all accumulations stay fp32.

Scale invariance: t is invariant to scaling s, so lhsT carries the raw
channel sums (not means) -> s' = HW*s, mu' = col[HW]/HW, var' accum
hwvar' = HW^3*var, rstd'' = rsqrt(hwvar' + HW^3*eps), and the host bakes
sqrt(HW) into the weight vector: a = (w*sqrt(HW)) * rstd''.

Port economics (TRN2): DVE's 2nd read port (needed by tensor_tensor and
by 2x_2P/4x packed single-src modes) is the SAME exclusive-lock port
pair GpSimd uses -- any GpSimd op head-of-line blocks a DVE TT.  And
tensor_reduce only has a 1x uop.  So:
  - channel sums: in-place tensor_scalar(*1.0, accum_out) on DVE for 3
    j's (single tensor operand; candidate for 2x_1P packing) + 1 j as
    in-place ACT Copy+accum
  - lhsT build: ACT Copy(m16, scale=sums_j) for 2 j's (ACT's dedicated
    port), DVE tensor_scalar for the other 2
  - rsqrt: DVE bit-trick + Newton, all [128,1] tensor_scalar with
    pointer scalars (dedicated port); squares on ACT.  Only
    {Copy, Identity, Square, Sigmoid} ACT funcs -> ONE table-set load.
  - gating muls: DVE TT for j0..2 (j0-1 fused with a broadcast gate),
    GpSimd TT for j3 emitted last so its shared-port hold overlaps
    DVE's dedicated-port work of the next batch.
"""

import sys

if "/opt/trn_rl_repo" not in sys.path:
    sys.path.insert(0, "/opt/trn_rl_repo")

from contextlib import ExitStack

import numpy as np
import ml_dtypes

import concourse.bacc as bacc
import concourse.bass as bass
import concourse.tile as tile
from concourse import mybir
from concourse.bass_utils import run_bass_kernel_spmd

B, C, H, W = 64, 512, 28, 28
G = 8
HW = H * W          # 784
NCORES = 8
BLOC = B // NCORES  # 8 batches per core
NP = 128            # SBUF partitions
NJ = C // NP        # 4 channel chunks per partition (c = NJ*p + j)
PBAND = NP // G     # 16 partitions per group
EPS = 1e-5
F32 = mybir.dt.float32
I32 = mybir.dt.int32
BF16 = mybir.dt.bfloat16
NPBF16 = np.dtype(ml_dtypes.bfloat16)
MMCHUNK = 512       # PSUM bank size in fp32 -> max matmul out free dim
RSQRT_MAGIC = 0x5F3759DF
HW3EPS = float(EPS) * HW * HW * HW

_cache: dict = {}

# implementation choices (bisectable)
OUT_ENGINE = "scalar"   # "scalar" or "sync" HWDGE ring for output DMAs
SUMS_MODE = "vvva"    # per-j engine for channel sums: v=DVE ts+accum,
                      # a=ACT copy+accum, r=DVE reduce,
                      # f=GpSimd TT-fold (x[0:392]+x[392:784]) + DVE tail
                      # NOTE: any GpSimd op degrades DVE TS/copy ops (shared
                      # port is needed for their 2x_2P mode) -- keep GpSimd idle
RSQRT_MODE = "nr"     # "nr": bit-trick+Newton ("pow" is not in the DVE ISA)
LHST_MODE = "aavv"    # per-j engine for lhsT build: a=ACT, v=DVE
MUL_MODE = "2v2v"     # "2vvg": DVE pair(j01)+single(j2), GpSimd j3
                      # "4v": single 4-row DVE TT with broadcast gate
                      # "2v2v": DVE two pairs; "2v2g": DVE pair + GpSimd pair
                      # "vvvv": 4 DVE singles
NR_ITERS = 1          # Newton iterations for rsqrt (bf16 error dominates)
PREF = 4              # input prefetch depth (batches)
SPLIT_IN_DMA = False  # one [128,4,784] load vs two halves


def _emit(tc, nc, xs, m8, wv, bv, ys):
    AF = mybir.ActivationFunctionType
    OP = mybir.AluOpType
    with ExitStack() as ctx:
        consts = ctx.enter_context(tc.tile_pool(name="consts", bufs=1))
        xpool = ctx.enter_context(tc.tile_pool(name="xpool", bufs=BLOC))
        mpool = ctx.enter_context(tc.tile_pool(name="mpool", bufs=3))
        vpool = ctx.enter_context(tc.tile_pool(name="vpool", bufs=4))
        gpool = ctx.enter_context(tc.tile_pool(name="gpool", bufs=4))
        spsum = ctx.enter_context(tc.tile_pool(name="spsum", bufs=4, space="PSUM"))
        opool = ctx.enter_context(tc.tile_pool(name="opool", bufs=3))

        xts = {}
        state = {}

        def dma_in(b):
            # cols HW:HW+2 later hold the raw channel sums so the matmul's
            # second chunk also accumulates HW^2*mu for free
            xt = xpool.tile([NP, NJ, HW + 2], BF16)
            if SPLIT_IN_DMA:
                nc.sync.dma_start(out=xt[:, 0:2, 0:HW], in_=xs[b, :, 0:2, :])
                nc.sync.dma_start(out=xt[:, 2:4, 0:HW], in_=xs[b, :, 2:4, :])
            else:
                nc.sync.dma_start(out=xt[:, :, 0:HW], in_=xs[b])
            xts[b] = xt

        HW2 = HW // 2
        NFOLD = SUMS_MODE.count("f")

        def phase1(b):
            xt = xts[b]
            sums = mpool.tile([NP, NJ], F32, tag="sums")
            if NFOLD:
                fold = mpool.tile([NP, NFOLD, HW2], BF16, tag="fold")
            for j, m in enumerate(SUMS_MODE):
                xj = xt[:, j, 0:HW]
                if m == "f":
                    fj = fold[:, j, :]
                    nc.gpsimd.tensor_add(
                        fj, xt[:, j, 0:HW2], xt[:, j, HW2:HW]
                    )
                    nc.vector.tensor_scalar(
                        out=fj, in0=fj, scalar1=1.0, scalar2=0.0,
                        op0=OP.mult, op1=OP.add,
                        accum_out=sums[:, j : j + 1],
                    )
                elif m == "v":
                    nc.vector.tensor_scalar(
                        out=xj, in0=xj, scalar1=1.0, scalar2=0.0,
                        op0=OP.mult, op1=OP.add,
                        accum_out=sums[:, j : j + 1],
                    )
                elif m == "a":
                    nc.scalar.activation(
                        out=xj, in_=xj, func=AF.Copy,
                        accum_out=sums[:, j : j + 1],
                    )
                elif m == "g":
                    nc.gpsimd.scalar_tensor_tensor(
                        out=xj, in0=xj, scalar=0.0, in1=xj,
                        op0=OP.mult, op1=OP.add,
                        accum_out=sums[:, j : j + 1],
                    )
                else:
                    nc.vector.reduce_sum(
                        out=sums[:, j : j + 1], in_=xj, axis=mybir.AxisListType.X
                    )
            # stash raw sums into the mu columns (bf16 cast)
            nc.vector.tensor_copy(
                xt[:, :, HW : HW + 2],
                sums[:].unsqueeze(2).to_broadcast([NP, NJ, 2]),
            )
            # lhsT[:, j, q] = m16[q in band(p)] * sums_j  (bf16)
            lhsT = mpool.tile([NP, NJ, NP], BF16, tag="lhsT")
            for j, m in enumerate(LHST_MODE):
                if m == "a":
                    nc.scalar.activation(
                        out=lhsT[:, j, :], in_=m16_sb[:], func=AF.Copy,
                        scale=sums[:, j : j + 1],
                    )
                else:
                    nc.vector.tensor_scalar(
                        out=lhsT[:, j, :], in0=m16_sb[:],
                        scalar1=sums[:, j : j + 1], scalar2=None, op0=OP.mult,
                    )
            state[b] = lhsT

        def phase2(b):
            # s' = HW*s (replicated per 16-band) in cols 0:HW; HW^2*mu in col HW
            xt = xts[b]
            lhsT = state[b]
            ps = spsum.tile([NP, HW + 2], F32)
            for c0 in range(0, HW + 2, MMCHUNK):
                c1 = min(c0 + MMCHUNK, HW + 2)
                for j in range(NJ):
                    nc.tensor.matmul(
                        ps[:, c0:c1], lhsT[:, j, :], xt[:, j, c0:c1],
                        start=(j == 0), stop=(j == NJ - 1),
                    )
            state[b] = ps

        pair_state = {}

        def phase3a(b):
            # per-batch stats: nmu and HW*var accumulated into pair tiles
            ps = state[b]
            k = b % 2
            if k == 0:
                nmu_p = vpool.tile([NP, 2], F32, tag="nmu_p")
                hwvar_p = vpool.tile([NP, 2], F32, tag="hwvar_p")
                pair_state[b // 2] = (nmu_p, hwvar_p, None, None)
            nmu_p, hwvar_p, _, _ = pair_state[b // 2]
            nc.scalar.activation(
                out=nmu_p[:, k : k + 1], in_=ps[:, HW : HW + 1], func=AF.Copy,
                scale=-1.0 / HW,
            )
            sq = gpool.tile([NP, HW], BF16, tag="sq")
            nc.scalar.activation(
                out=sq[:], in_=ps[:, 0:HW], func=AF.Square,
                bias=nmu_p[:, k : k + 1], accum_out=hwvar_p[:, k : k + 1],
            )

        def phase3b(p):
            # pair-batched rsqrt: bit-trick seed + Newton on [NP, 2]
            # (eps dropped: v = HW^3*(var+~0) and var >> eps for this data)
            nmu_p, hwvar_p, _, _ = pair_state[p]
            a_t = vpool.tile([NP, 2], F32, tag="a_t")
            if RSQRT_MODE == "pow":
                # a = w*sqrt(HW) * v^-0.5 in a single DVE op
                nc.vector.tensor_scalar(
                    out=a_t[:], in0=hwvar_p[:], scalar1=-0.5,
                    scalar2=wv_sb[:, 0:1], op0=OP.pow, op1=OP.mult,
                )
            else:
                y_t = vpool.tile([NP, 2], F32, tag="y_t")
                nc.vector.tensor_scalar(
                    out=y_t[:].bitcast(I32), in0=hwvar_p[:].bitcast(I32),
                    scalar1=1, scalar2=-1,
                    op0=OP.logical_shift_right, op1=OP.bitwise_xor,
                )
                nc.vector.tensor_scalar_add(
                    y_t[:].bitcast(I32), y_t[:].bitcast(I32), RSQRT_MAGIC + 1
                )
                # Newton: y <- y * (1.5 - 0.5*v*y^2); last mul folds w*sqrt(HW)
                t1 = vpool.tile([NP, 2], F32, tag="t1")
                u_t = vpool.tile([NP, 2], F32, tag="u_t")
                y = y_t
                for it in range(NR_ITERS):
                    nc.scalar.activation(out=t1[:], in_=y[:], func=AF.Square)
                    nc.vector.scalar_tensor_tensor(
                        out=u_t[:], in0=t1[:], scalar=-0.5, in1=hwvar_p[:],
                        op0=OP.mult, op1=OP.mult,
                    )
                    nc.vector.tensor_scalar_add(u_t[:], u_t[:], 1.5)
                    if it < NR_ITERS - 1:
                        yn = vpool.tile([NP, 2], F32, tag=f"y{it}")
                        nc.vector.tensor_mul(yn[:], y[:], u_t[:])
                        y = yn
                nc.vector.scalar_tensor_tensor(
                    out=a_t[:], in0=y[:], scalar=wv_sb[:, 0:1], in1=u_t[:],
                    op0=OP.mult, op1=OP.mult,
                )
            c_t = vpool.tile([NP, 2], F32, tag="c_t")
            nc.vector.tensor_mul(c_t[:], nmu_p[:], a_t[:])
            nc.vector.tensor_scalar(
                out=c_t[:], in0=c_t[:], scalar1=bv_sb[:, 0:1], scalar2=None,
                op0=OP.add,
            )
            pair_state[p] = (nmu_p, hwvar_p, a_t, c_t)

        def phase3c(b):
            # per-batch gate from the pair's a/c columns
            ps = state[b]
            k = b % 2
            _, _, a_t, c_t = pair_state[b // 2]
            gate = gpool.tile([NP, HW], BF16, tag="gate")
            nc.scalar.activation(
                out=gate[:], in_=ps[:, 0:HW], func=AF.Sigmoid,
                bias=c_t[:, k : k + 1], scale=a_t[:, k : k + 1],
            )
            state[b] = gate

        dma_eng = nc.scalar if OUT_ENGINE == "scalar" else nc.sync

        def pairmul(eng, ot, xt, gate, j0):
            eng.tensor_mul(
                ot[:, j0 : j0 + 2, :], xt[:, j0 : j0 + 2, 0:HW],
                gate[:].unsqueeze(1).to_broadcast([NP, 2, HW]),
            )

        def phase4a(b):
            # first half of the gating multiply + store of j0/j1
            xt = xts[b]
            gate = state[b]
            ot = opool.tile([NP, NJ, HW], BF16)
            state[b] = (gate, ot)
            if MUL_MODE == "4v":
                nc.vector.tensor_mul(
                    ot[:], xt[:, :, 0:HW],
                    gate[:].unsqueeze(1).to_broadcast([NP, NJ, HW]),
                )
                dma_eng.dma_start(out=ys[b], in_=ot[:])
                return
            if MUL_MODE == "vvvv":
                nc.vector.tensor_mul(ot[:, 0, :], xt[:, 0, 0:HW], gate[:])
                nc.vector.tensor_mul(ot[:, 1, :], xt[:, 1, 0:HW], gate[:])
            else:
                pairmul(nc.vector, ot, xt, gate, 0)
            dma_eng.dma_start(out=ys[b, :, 0:2, :], in_=ot[:, 0:2, :])

        def phase4b(b):
            # second half (j2/j3) + store; GpSimd op (if any) emitted first
            xt = xts.pop(b)
            gate, ot = state.pop(b)
            if MUL_MODE == "4v":
                if b + PREF < BLOC:
                    dma_in(b + PREF)
                return
            if MUL_MODE == "2vvg":
                nc.gpsimd.tensor_mul(ot[:, 3, :], xt[:, 3, 0:HW], gate[:])
                nc.vector.tensor_mul(ot[:, 2, :], xt[:, 2, 0:HW], gate[:])
            elif MUL_MODE == "2v2v":
                pairmul(nc.vector, ot, xt, gate, 2)
            elif MUL_MODE == "2v2g":
                pairmul(nc.gpsimd, ot, xt, gate, 2)
            else:
                nc.vector.tensor_mul(ot[:, 2, :], xt[:, 2, 0:HW], gate[:])
                nc.vector.tensor_mul(ot[:, 3, :], xt[:, 3, 0:HW], gate[:])
            dma_eng.dma_start(out=ys[b, :, 2:4, :], in_=ot[:, 2:4, :])
            if b + PREF < BLOC:
                dma_in(b + PREF)

        # software-pipelined emission: each engine's stream sees work in
        # data-readiness order, so in-order engines never head-of-line block.
        # The first x tile is on the critical path; consts go after it.
        dma_in(0)
        # m8 carries the [NP, NP] block-banded 0/1 indicator
        # M16[p, q] = (p//PBAND == q//PBAND); wv (= w*sqrt(HW)) and bv are
        # 16x-replicated [NP, 1]
        m16_sb = consts.tile([NP, NP], BF16)
        nc.sync.dma_start(out=m16_sb[:], in_=m8[:])
        wv_sb = consts.tile([NP, 1], F32)
        nc.sync.dma_start(out=wv_sb[:], in_=wv[:])
        bv_sb = consts.tile([NP, 1], F32)
        nc.sync.dma_start(out=bv_sb[:], in_=bv[:])
        for b in range(1, min(PREF, BLOC)):
            dma_in(b)
        phase1(0)
        phase2(0)
        phase1(1)
        phase2(1)
        for p in range(BLOC // 2):
            b0, b1 = 2 * p, 2 * p + 1
            phase3a(b0)
            if b0 + 2 < BLOC:
                phase1(b0 + 2)
            phase3a(b1)
            if b0 + 2 < BLOC:
                phase2(b0 + 2)
            phase3b(p)
            phase3c(b0)
            phase4a(b0)
            if b1 + 2 < BLOC:
                phase1(b1 + 2)
            phase4b(b0)
            phase3c(b1)
            phase4a(b1)
            if b1 + 2 < BLOC:
                phase2(b1 + 2)
            phase4b(b1)


def _build_nc():
    nc = bacc.Bacc("TRN2", debug=False)
    xs = nc.dram_tensor("xs", [BLOC, NP, NJ, HW], BF16, kind="ExternalInput")
    m8 = nc.dram_tensor("m8", [NP, NP], BF16, kind="ExternalInput")
    wv = nc.dram_tensor("wv", [NP, 1], F32, kind="ExternalInput")
    bv = nc.dram_tensor("bv", [NP, 1], F32, kind="ExternalInput")
    ys = nc.dram_tensor("ys", [BLOC, NP, NJ, HW], BF16, kind="ExternalOutput")
    with tile.TileContext(nc) as tc:
        _emit(tc, nc, xs, m8, wv, bv, ys)
    nc.compile()
    return nc


def get_nc():
    if "nc" not in _cache:
        _cache["nc"] = _build_nc()
    return _cache["nc"]


def make_in_maps(x, weight, bias):
    x = np.ascontiguousarray(np.asarray(x, dtype=np.float32))
    weight = np.asarray(weight, dtype=np.float32).reshape(G)
    bias = np.asarray(bias, dtype=np.float32).reshape(G)
    # [core, b, p, j, hw] with c = NJ*p + j; downcast to bf16 on host
    xs = x.reshape(NCORES, BLOC, NP, NJ, HW).astype(NPBF16)
    band = np.arange(NP) // PBAND
    m8 = (band[:, None] == band[None, :]).astype(NPBF16)  # [NP, NP] indicator
    wv = np.ascontiguousarray(
        (np.repeat(weight, PBAND) * np.sqrt(float(HW)))[:, None]
    ).astype(np.float32)
    bv = np.ascontiguousarray(np.repeat(bias, PBAND)[:, None])
    return [
        {"xs": np.ascontiguousarray(xs[i]), "m8": m8, "wv": wv, "bv": bv}
        for i in range(NCORES)
    ]


def run(x, weight, bias, trace=False, **spmd_kwargs):
    nc = get_nc()
    in_maps = make_in_maps(x, weight, bias)
    res = run_bass_kernel_spmd(
        nc, in_maps, core_ids=list(range(NCORES)), trace=trace, **spmd_kwargs
    )
    out = np.stack(
        [res.results[i]["ys"].astype(np.float32) for i in range(NCORES)]
    )
    return out.reshape(B, C, H, W), res


def kernel(x, weight, bias, groups=G, **_ignored):
    assert int(groups) == G
    out, _ = run(x, weight, bias, trace=False)
    return out


# revision 36
# speedup vs baseline: 1.0280x; 1.0280x over previous
"""Trainium2 Bass kernel: grouped similarity-gating normalization (bf16 I/O).

Reference computation (per batch b, group g, cpg=64 channels, hw=784):
    means[c]  = mean_hw(x[c, :])
    s[hw]     = sum_c x[c, hw] * means[c]
    t         = (s - mean(s)) * rsqrt(var(s) + eps)
    gate      = sigmoid(t * weight[g] + bias[g])
    out[c,hw] = x[c, hw] * gate[hw]

Sharding: data-parallel over batch B=64 across 8 cores (8 batches/core).
Harness gate is rel_err < 2e-2; x is bf16 on the wire (halves HBM traffic
-> ~36us DMA roofline/core), all accumulations stay fp32.

Scale invariance: t is invariant to scaling s, so lhsT carries the raw
channel sums (not means) -> s' = HW*s, mu' = col[HW]/HW, var' accum
hwvar' = HW^3*var, rstd'' = rsqrt(hwvar' + HW^3*eps), and the host bakes
sqrt(HW) into the weight vector: a = (w*sqrt(HW)) * rstd''.

Port economics (TRN2): DVE's 2nd read port (needed by tensor_tensor and
by 2x_2P/4x packed single-src modes) is the SAME exclusive-lock port
pair GpSimd uses -- any GpSimd op degrades concurrent DVE TT/TS ops
(measured: TS 214ns -> 434ns with GpSimd active), so GpSimd is kept
IDLE.  tensor_reduce only has a 1x uop; TS+accum (TensorScalarCacheReduce)
is also 1x.  Final engine split, per batch:
  - channel sums: in-place tensor_scalar(*1.0, accum_out) on DVE for
    j0..2 (962ns each), in-place ACT Copy+accum for j3
  - lhsT build: ACT Copy(m16, scale=sums_j) for j0/j1, DVE TS for j2/j3
  - stats: Square/Sigmoid on ACT reading PSUM directly; rsqrt via DVE
    bit-trick + 1 Newton iteration, batched over PAIRS of batches on
    [128,2] tiles (bf16 error dominates; eps is negligible vs var).
    Only {Copy, Identity, Square, Sigmoid} ACT funcs -> ONE table load.
  - gating muls: two DVE TT pair-ops [128,2,784] with broadcast gate
    (2x_1P, ~974ns each); output DMA split j01/j23 to start stores early.
Measured: 126.0us (fp32 baseline) -> 67.6us.  Vector stream ~63us is
the bottleneck (sums 23us + gating TT 16us + fixed ~150ns/instr);
Vector+ACT combined work bounds this structure at ~58us.
"""

import sys

if "/opt/trn_rl_repo" not in sys.path:
    sys.path.insert(0, "/opt/trn_rl_repo")

from contextlib import ExitStack

import numpy as np
import ml_dtypes

import concourse.bacc as bacc
import concourse.bass as bass
import concourse.tile as tile
from concourse import mybir
from concourse.bass_utils import run_bass_kernel_spmd

B, C, H, W = 64, 512, 28, 28
G = 8
HW = H * W          # 784
NCORES = 8
BLOC = B // NCORES  # 8 batches per core
NP = 128            # SBUF partitions
NJ = C // NP        # 4 channel chunks per partition (c = NJ*p + j)
PBAND = NP // G     # 16 partitions per group
EPS = 1e-5
F32 = mybir.dt.float32
I32 = mybir.dt.int32
BF16 = mybir.dt.bfloat16
NPBF16 = np.dtype(ml_dtypes.bfloat16)
MMCHUNK = 512       # PSUM bank size in fp32 -> max matmul out free dim
RSQRT_MAGIC = 0x5F3759DF
HW3EPS = float(EPS) * HW * HW * HW

_cache: dict = {}

# implementation choices (bisectable)
OUT_ENGINE = "sync"   # "scalar" or "sync" HWDGE ring for output DMAs
SUMS_MODE = "vvva"    # per-j engine for channel sums: v=DVE ts+accum,
                      # a=ACT copy+accum, r=DVE reduce,
                      # f=GpSimd TT-fold (x[0:392]+x[392:784]) + DVE tail
                      # NOTE: any GpSimd op degrades DVE TS/copy ops (shared
                      # port is needed for their 2x_2P mode) -- keep GpSimd idle
RSQRT_MODE = "nr"     # "nr": bit-trick+Newton ("pow" is not in the DVE ISA)
LHST_MODE = "aavv"    # per-j engine for lhsT build: a=ACT, v=DVE
MUL_MODE = "2v2v"     # "2vvg": DVE pair(j01)+single(j2), GpSimd j3
                      # "4v": single 4-row DVE TT with broadcast gate
                      # "2v2v": DVE two pairs; "2v2g": DVE pair + GpSimd pair
                      # "vvvv": 4 DVE singles
NR_ITERS = 1          # Newton iterations for rsqrt (bf16 error dominates)
PREF = 4              # input prefetch depth (batches)
SPLIT_IN_DMA = False  # one [128,4,784] load vs two halves


def _emit(tc, nc, xs, m8, wv, bv, ys):
    AF = mybir.ActivationFunctionType
    OP = mybir.AluOpType
    with ExitStack() as ctx:
        consts = ctx.enter_context(tc.tile_pool(name="consts", bufs=1))
        xpool = ctx.enter_context(tc.tile_pool(name="xpool", bufs=BLOC))
        mpool = ctx.enter_context(tc.tile_pool(name="mpool", bufs=3))
        vpool = ctx.enter_context(tc.tile_pool(name="vpool", bufs=4))
        gpool = ctx.enter_context(tc.tile_pool(name="gpool", bufs=4))
        spsum = ctx.enter_context(tc.tile_pool(name="spsum", bufs=4, space="PSUM"))
        opool = ctx.enter_context(tc.tile_pool(name="opool", bufs=3))

        xts = {}
        state = {}

        def dma_in(b):
            # cols HW:HW+2 later hold the raw channel sums so the matmul's
            # second chunk also accumulates HW^2*mu for free
            xt = xpool.tile([NP, NJ, HW + 2], BF16)
            if SPLIT_IN_DMA:
                nc.sync.dma_start(out=xt[:, 0:2, 0:HW], in_=xs[b, :, 0:2, :])
                nc.sync.dma_start(out=xt[:, 2:4, 0:HW], in_=xs[b, :, 2:4, :])
            else:
                nc.sync.dma_start(out=xt[:, :, 0:HW], in_=xs[b])
            xts[b] = xt

        HW2 = HW // 2
        NFOLD = SUMS_MODE.count("f")

        def phase1(b):
            xt = xts[b]
            sums = mpool.tile([NP, NJ], F32, tag="sums")
            if NFOLD:
                fold = mpool.tile([NP, NFOLD, HW2], BF16, tag="fold")
            for j, m in enumerate(SUMS_MODE):
                xj = xt[:, j, 0:HW]
                if m == "f":
                    fj = fold[:, j, :]
                    nc.gpsimd.tensor_add(
                        fj, xt[:, j, 0:HW2], xt[:, j, HW2:HW]
                    )
                    nc.vector.tensor_scalar(
                        out=fj, in0=fj, scalar1=1.0, scalar2=0.0,
                        op0=OP.mult, op1=OP.add,
                        accum_out=sums[:, j : j + 1],
                    )
                elif m == "v":
                    nc.vector.tensor_scalar(
                        out=xj, in0=xj, scalar1=1.0, scalar2=0.0,
                        op0=OP.mult, op1=OP.add,
                        accum_out=sums[:, j : j + 1],
                    )
                elif m == "a":
                    nc.scalar.activation(
                        out=xj, in_=xj, func=AF.Copy,
                        accum_out=sums[:, j : j + 1],
                    )
                elif m == "g":
                    nc.gpsimd.scalar_tensor_tensor(
                        out=xj, in0=xj, scalar=0.0, in1=xj,
                        op0=OP.mult, op1=OP.add,
                        accum_out=sums[:, j : j + 1],
                    )
                else:
                    nc.vector.reduce_sum(
                        out=sums[:, j : j + 1], in_=xj, axis=mybir.AxisListType.X
                    )
            # stash raw sums into the mu columns (bf16 cast)
            nc.vector.tensor_copy(
                xt[:, :, HW : HW + 2],
                sums[:].unsqueeze(2).to_broadcast([NP, NJ, 2]),
            )
            # lhsT[:, j, q] = m16[q in band(p)] * sums_j  (bf16)
            lhsT = mpool.tile([NP, NJ, NP], BF16, tag="lhsT")
            for j, m in enumerate(LHST_MODE):
                if m == "a":
                    nc.scalar.activation(
                        out=lhsT[:, j, :], in_=m16_sb[:], func=AF.Copy,
                        scale=sums[:, j : j + 1],
                    )
                else:
                    nc.vector.tensor_scalar(
                        out=lhsT[:, j, :], in0=m16_sb[:],
                        scalar1=sums[:, j : j + 1], scalar2=None, op0=OP.mult,
                    )
            state[b] = lhsT

        def phase2(b):
            # s' = HW*s (replicated per 16-band) in cols 0:HW; HW^2*mu in col HW
            xt = xts[b]
            lhsT = state[b]
            ps = spsum.tile([NP, HW + 2], F32)
            for c0 in range(0, HW + 2, MMCHUNK):
                c1 = min(c0 + MMCHUNK, HW + 2)
                for j in range(NJ):
                    nc.tensor.matmul(
                        ps[:, c0:c1], lhsT[:, j, :], xt[:, j, c0:c1],
                        start=(j == 0), stop=(j == NJ - 1),
                    )
            state[b] = ps

        pair_state = {}

        def phase3a(b):
            # per-batch stats: nmu and HW*var accumulated into pair tiles
            ps = state[b]
            k = b % 2
            if k == 0:
                nmu_p = vpool.tile([NP, 2], F32, tag="nmu_p")
                hwvar_p = vpool.tile([NP, 2], F32, tag="hwvar_p")
                pair_state[b // 2] = (nmu_p, hwvar_p, None, None)
            nmu_p, hwvar_p, _, _ = pair_state[b // 2]
            nc.scalar.activation(
                out=nmu_p[:, k : k + 1], in_=ps[:, HW : HW + 1], func=AF.Copy,
                scale=-1.0 / HW,
            )
            sq = gpool.tile([NP, HW], BF16, tag="sq")
            nc.scalar.activation(
                out=sq[:], in_=ps[:, 0:HW], func=AF.Square,
                bias=nmu_p[:, k : k + 1], accum_out=hwvar_p[:, k : k + 1],
            )

        def phase3b(p):
            # pair-batched rsqrt: bit-trick seed + Newton on [NP, 2]
            # (eps dropped: v = HW^3*(var+~0) and var >> eps for this data)
            nmu_p, hwvar_p, _, _ = pair_state[p]
            a_t = vpool.tile([NP, 2], F32, tag="a_t")
            if RSQRT_MODE == "pow":
                # a = w*sqrt(HW) * v^-0.5 in a single DVE op
                nc.vector.tensor_scalar(
                    out=a_t[:], in0=hwvar_p[:], scalar1=-0.5,
                    scalar2=wv_sb[:, 0:1], op0=OP.pow, op1=OP.mult,
                )
            else:
                y_t = vpool.tile([NP, 2], F32, tag="y_t")
                nc.vector.tensor_scalar(
                    out=y_t[:].bitcast(I32), in0=hwvar_p[:].bitcast(I32),
                    scalar1=1, scalar2=-1,
                    op0=OP.logical_shift_right, op1=OP.bitwise_xor,
                )
                nc.vector.tensor_scalar_add(
                    y_t[:].bitcast(I32), y_t[:].bitcast(I32), RSQRT_MAGIC + 1
                )
                # Newton: y <- y * (1.5 - 0.5*v*y^2); last mul folds w*sqrt(HW)
                t1 = vpool.tile([NP, 2], F32, tag="t1")
                u_t = vpool.tile([NP, 2], F32, tag="u_t")
                y = y_t
                for it in range(NR_ITERS):
                    nc.scalar.activation(out=t1[:], in_=y[:], func=AF.Square)
                    nc.vector.scalar_tensor_tensor(
                        out=u_t[:], in0=t1[:], scalar=-0.5, in1=hwvar_p[:],
                        op0=OP.mult, op1=OP.mult,
                    )
                    nc.vector.tensor_scalar_add(u_t[:], u_t[:], 1.5)
                    if it < NR_ITERS - 1:
                        yn = vpool.tile([NP, 2], F32, tag=f"y{it}")
                        nc.vector.tensor_mul(yn[:], y[:], u_t[:])
                        y = yn
                nc.vector.scalar_tensor_tensor(
                    out=a_t[:], in0=y[:], scalar=wv_sb[:, 0:1], in1=u_t[:],
                    op0=OP.mult, op1=OP.mult,
                )
            c_t = vpool.tile([NP, 2], F32, tag="c_t")
            nc.vector.tensor_mul(c_t[:], nmu_p[:], a_t[:])
            nc.vector.tensor_scalar(
                out=c_t[:], in0=c_t[:], scalar1=bv_sb[:, 0:1], scalar2=None,
                op0=OP.add,
            )
            pair_state[p] = (nmu_p, hwvar_p, a_t, c_t)

        def phase3c(b):
            # per-batch gate from the pair's a/c columns
            ps = state[b]
            k = b % 2
            _, _, a_t, c_t = pair_state[b // 2]
            gate = gpool.tile([NP, HW], BF16, tag="gate")
            nc.scalar.activation(
                out=gate[:], in_=ps[:, 0:HW], func=AF.Sigmoid,
                bias=c_t[:, k : k + 1], scale=a_t[:, k : k + 1],
            )
            state[b] = gate

        dma_eng = nc.scalar if OUT_ENGINE == "scalar" else nc.sync

        def pairmul(eng, ot, xt, gate, j0):
            eng.tensor_mul(
                ot[:, j0 : j0 + 2, :], xt[:, j0 : j0 + 2, 0:HW],
                gate[:].unsqueeze(1).to_broadcast([NP, 2, HW]),
            )

        def phase4a(b):
            # first half of the gating multiply + store of j0/j1
            xt = xts[b]
            gate = state[b]
            ot = opool.tile([NP, NJ, HW], BF16)
            state[b] = (gate, ot)
            if MUL_MODE == "4v":
                nc.vector.tensor_mul(
                    ot[:], xt[:, :, 0:HW],
                    gate[:].unsqueeze(1).to_broadcast([NP, NJ, HW]),
                )
                dma_eng.dma_start(out=ys[b], in_=ot[:])
                return
            if MUL_MODE == "vvvv":
                nc.vector.tensor_mul(ot[:, 0, :], xt[:, 0, 0:HW], gate[:])
                nc.vector.tensor_mul(ot[:, 1, :], xt[:, 1, 0:HW], gate[:])
            else:
                pairmul(nc.vector, ot, xt, gate, 0)
            dma_eng.dma_start(out=ys[b, :, 0:2, :], in_=ot[:, 0:2, :])

        def phase4b(b):
            # second half (j2/j3) + store; GpSimd op (if any) emitted first
            xt = xts.pop(b)
            gate, ot = state.pop(b)
            if MUL_MODE == "4v":
                if b + PREF < BLOC:
                    dma_in(b + PREF)
                return
            if MUL_MODE == "2vvg":
                nc.gpsimd.tensor_mul(ot[:, 3, :], xt[:, 3, 0:HW], gate[:])
                nc.vector.tensor_mul(ot[:, 2, :], xt[:, 2, 0:HW], gate[:])
            elif MUL_MODE == "2v2v":
                pairmul(nc.vector, ot, xt, gate, 2)
            elif MUL_MODE == "2v2g":
                pairmul(nc.gpsimd, ot, xt, gate, 2)
            else:
                nc.vector.tensor_mul(ot[:, 2, :], xt[:, 2, 0:HW], gate[:])
                nc.vector.tensor_mul(ot[:, 3, :], xt[:, 3, 0:HW], gate[:])
            dma_eng.dma_start(out=ys[b, :, 2:4, :], in_=ot[:, 2:4, :])
            if b + PREF < BLOC:
                dma_in(b + PREF)

        # software-pipelined emission: each engine's stream sees work in
        # data-readiness order, so in-order engines never head-of-line block.
        # The first x tile is on the critical path; consts go after it.
        dma_in(0)
        # m8 carries the [NP, NP] block-banded 0/1 indicator
        # M16[p, q] = (p//PBAND == q//PBAND); wv (= w*sqrt(HW)) and bv are
        # 16x-replicated [NP, 1]
        m16_sb = consts.tile([NP, NP], BF16)
        nc.sync.dma_start(out=m16_sb[:], in_=m8[:])
        wv_sb = consts.tile([NP, 1], F32)
        nc.sync.dma_start(out=wv_sb[:], in_=wv[:])
        bv_sb = consts.tile([NP, 1], F32)
        nc.sync.dma_start(out=bv_sb[:], in_=bv[:])
        for b in range(1, min(PREF, BLOC)):
            dma_in(b)
        phase1(0)
        phase2(0)
        phase1(1)
        phase2(1)
        for p in range(BLOC // 2):
            b0, b1 = 2 * p, 2 * p + 1
            phase3a(b0)
            if b0 + 2 < BLOC:
                phase1(b0 + 2)
            phase3a(b1)
            if b0 + 2 < BLOC:
                phase2(b0 + 2)
            phase3b(p)
            phase3c(b0)
            phase4a(b0)
            if b1 + 2 < BLOC:
                phase1(b1 + 2)
            phase4b(b0)
            phase3c(b1)
            phase4a(b1)
            if b1 + 2 < BLOC:
                phase2(b1 + 2)
            phase4b(b1)


def _build_nc():
    nc = bacc.Bacc("TRN2", debug=False)
    xs = nc.dram_tensor("xs", [BLOC, NP, NJ, HW], BF16, kind="ExternalInput")
    m8 = nc.dram_tensor("m8", [NP, NP], BF16, kind="ExternalInput")
    wv = nc.dram_tensor("wv", [NP, 1], F32, kind="ExternalInput")
    bv = nc.dram_tensor("bv", [NP, 1], F32, kind="ExternalInput")
    ys = nc.dram_tensor("ys", [BLOC, NP, NJ, HW], BF16, kind="ExternalOutput")
    with tile.TileContext(nc) as tc:
        _emit(tc, nc, xs, m8, wv, bv, ys)
    nc.compile()
    return nc


def get_nc():
    if "nc" not in _cache:
        _cache["nc"] = _build_nc()
    return _cache["nc"]


def make_in_maps(x, weight, bias):
    x = np.ascontiguousarray(np.asarray(x, dtype=np.float32))
    weight = np.asarray(weight, dtype=np.float32).reshape(G)
    bias = np.asarray(bias, dtype=np.float32).reshape(G)
    # [core, b, p, j, hw] with c = NJ*p + j; downcast to bf16 on host
    xs = x.reshape(NCORES, BLOC, NP, NJ, HW).astype(NPBF16)
    band = np.arange(NP) // PBAND
    m8 = (band[:, None] == band[None, :]).astype(NPBF16)  # [NP, NP] indicator
    wv = np.ascontiguousarray(
        (np.repeat(weight, PBAND) * np.sqrt(float(HW)))[:, None]
    ).astype(np.float32)
    bv = np.ascontiguousarray(np.repeat(bias, PBAND)[:, None])
    return [
        {"xs": np.ascontiguousarray(xs[i]), "m8": m8, "wv": wv, "bv": bv}
        for i in range(NCORES)
    ]


def run(x, weight, bias, trace=False, **spmd_kwargs):
    nc = get_nc()
    in_maps = make_in_maps(x, weight, bias)
    res = run_bass_kernel_spmd(
        nc, in_maps, core_ids=list(range(NCORES)), trace=trace, **spmd_kwargs
    )
    out = np.stack(
        [res.results[i]["ys"].astype(np.float32) for i in range(NCORES)]
    )
    return out.reshape(B, C, H, W), res


def kernel(x, weight, bias, groups=G, **_ignored):
    assert int(groups) == G
    out, _ = run(x, weight, bias, trace=False)
    return out


# revision 37
# speedup vs baseline: 1.0830x; 1.0535x over previous
"""Trainium2 Bass kernel: grouped similarity-gating normalization (bf16 I/O).

Reference computation (per batch b, group g, cpg=64 channels, hw=784):
    means[c]  = mean_hw(x[c, :])
    s[hw]     = sum_c x[c, hw] * means[c]
    t         = (s - mean(s)) * rsqrt(var(s) + eps)
    gate      = sigmoid(t * weight[g] + bias[g])
    out[c,hw] = x[c, hw] * gate[hw]

Sharding: data-parallel over batch B=64 across 8 cores (8 batches/core).
Harness gate is rel_err < 2e-2; x is bf16 on the wire (halves HBM traffic
-> ~36us DMA roofline/core), all accumulations stay fp32.

Scale invariance: t is invariant to scaling s, so lhsT carries the raw
channel sums (not means) -> s' = HW*s, mu' = col[HW]/HW, var' accum
hwvar' = HW^3*var, rstd'' = rsqrt(hwvar' + HW^3*eps), and the host bakes
sqrt(HW) into the weight vector: a = (w*sqrt(HW)) * rstd''.

Port economics (TRN2): DVE's 2nd read port (needed by tensor_tensor and
by 2x_2P/4x packed single-src modes) is the SAME exclusive-lock port
pair GpSimd uses -- any GpSimd op degrades concurrent DVE TT/TS ops
(measured: TS 214ns -> 434ns with GpSimd active), so GpSimd is kept
IDLE.  tensor_reduce only has a 1x uop; TS+accum (TensorScalarCacheReduce)
is also 1x.  Final engine split, per batch:
  - channel sums: in-place tensor_scalar(*1.0, accum_out) on DVE for
    j0..2 (962ns each), in-place ACT Copy+accum for j3
  - lhsT build: ACT Copy(m16, scale=sums_j) for j0/j1, DVE TS for j2/j3
  - stats: Square/Sigmoid on ACT reading PSUM directly; rsqrt via DVE
    bit-trick + 1 Newton iteration, batched over PAIRS of batches on
    [128,2] tiles (bf16 error dominates; eps is negligible vs var).
    Only {Copy, Identity, Square, Sigmoid} ACT funcs -> ONE table load.
  - gating muls: two DVE TT pair-ops [128,2,784] with broadcast gate
    (2x_1P, ~974ns each); output DMA split j01/j23 to start stores early.
Measured: 126.0us (fp32 baseline) -> 67.6us.  Vector stream ~63us is
the bottleneck (sums 23us + gating TT 16us + fixed ~150ns/instr);
Vector+ACT combined work bounds this structure at ~58us.
"""

import sys

if "/opt/trn_rl_repo" not in sys.path:
    sys.path.insert(0, "/opt/trn_rl_repo")

from contextlib import ExitStack

import numpy as np
import ml_dtypes

import concourse.bacc as bacc
import concourse.bass as bass
import concourse.tile as tile
from concourse import mybir
from concourse.bass_utils import run_bass_kernel_spmd

B, C, H, W = 64, 512, 28, 28
G = 8
HW = H * W          # 784
NCORES = 8
BLOC = B // NCORES  # 8 batches per core
NP = 128            # SBUF partitions
NJ = C // NP        # 4 channel chunks per partition (c = NJ*p + j)
PBAND = NP // G     # 16 partitions per group
EPS = 1e-5
F32 = mybir.dt.float32
I32 = mybir.dt.int32
BF16 = mybir.dt.bfloat16
NPBF16 = np.dtype(ml_dtypes.bfloat16)
MMCHUNK = 512       # PSUM bank size in fp32 -> max matmul out free dim
RSQRT_MAGIC = 0x5F3759DF
HW3EPS = float(EPS) * HW * HW * HW

_cache: dict = {}

# implementation choices (bisectable)
OUT_ENGINE = "sync"   # "scalar" or "sync" HWDGE ring for output DMAs
SUMS_MODE = "vvva"    # per-j engine for channel sums: v=DVE ts+accum,
                      # a=ACT copy+accum, r=DVE reduce,
                      # f=GpSimd TT-fold (x[0:392]+x[392:784]) + DVE tail
                      # NOTE: any GpSimd op degrades DVE TS/copy ops (shared
                      # port is needed for their 2x_2P mode) -- keep GpSimd idle
RSQRT_MODE = "nr"     # "nr": bit-trick+Newton ("pow" is not in the DVE ISA)
LHST_MODE = "aavv"    # per-j engine for lhsT build: a=ACT, v=DVE
MUL_MODE = "2v2v"     # "2vvg": DVE pair(j01)+single(j2), GpSimd j3
                      # "4v": single 4-row DVE TT with broadcast gate
                      # "2v2v": DVE two pairs; "2v2g": DVE pair + GpSimd pair
                      # "vvvv": 4 DVE singles
NR_ITERS = 1          # Newton iterations for rsqrt (bf16 error dominates)
PREF = 4              # input prefetch depth (batches)
SPLIT_IN_DMA = False  # one [128,4,784] load vs two halves


def _emit(tc, nc, xs, m8, wv, bv, ys):
    AF = mybir.ActivationFunctionType
    OP = mybir.AluOpType
    with ExitStack() as ctx:
        consts = ctx.enter_context(tc.tile_pool(name="consts", bufs=1))
        xpool = ctx.enter_context(tc.tile_pool(name="xpool", bufs=BLOC))
        mpool = ctx.enter_context(tc.tile_pool(name="mpool", bufs=3))
        vpool = ctx.enter_context(tc.tile_pool(name="vpool", bufs=4))
        gpool = ctx.enter_context(tc.tile_pool(name="gpool", bufs=4))
        spsum = ctx.enter_context(tc.tile_pool(name="spsum", bufs=4, space="PSUM"))
        opool = ctx.enter_context(tc.tile_pool(name="opool", bufs=3))

        xts = {}
        state = {}

        def dma_in(b):
            # cols HW:HW+2 later hold the raw channel sums so the matmul's
            # second chunk also accumulates HW^2*mu for free
            xt = xpool.tile([NP, NJ, HW + 2], BF16)
            if SPLIT_IN_DMA:
                nc.sync.dma_start(out=xt[:, 0:2, 0:HW], in_=xs[b, :, 0:2, :])
                nc.sync.dma_start(out=xt[:, 2:4, 0:HW], in_=xs[b, :, 2:4, :])
            else:
                nc.sync.dma_start(out=xt[:, :, 0:HW], in_=xs[b])
            xts[b] = xt

        HW2 = HW // 2
        NFOLD = SUMS_MODE.count("f")

        def phase1(b):
            xt = xts[b]
            sums = mpool.tile([NP, NJ], F32, tag="sums")
            if NFOLD:
                fold = mpool.tile([NP, NFOLD, HW2], BF16, tag="fold")
            for j, m in enumerate(SUMS_MODE):
                xj = xt[:, j, 0:HW]
                if m == "f":
                    fj = fold[:, j, :]
                    nc.gpsimd.tensor_add(
                        fj, xt[:, j, 0:HW2], xt[:, j, HW2:HW]
                    )
                    nc.vector.tensor_scalar(
                        out=fj, in0=fj, scalar1=1.0, scalar2=0.0,
                        op0=OP.mult, op1=OP.add,
                        accum_out=sums[:, j : j + 1],
                    )
                elif m == "v":
                    nc.vector.tensor_scalar(
                        out=xj, in0=xj, scalar1=1.0, scalar2=0.0,
                        op0=OP.mult, op1=OP.add,
                        accum_out=sums[:, j : j + 1],
                    )
                elif m == "a":
                    nc.scalar.activation(
                        out=xj, in_=xj, func=AF.Copy,
                        accum_out=sums[:, j : j + 1],
                    )
                elif m == "g":
                    nc.gpsimd.scalar_tensor_tensor(
                        out=xj, in0=xj, scalar=0.0, in1=xj,
                        op0=OP.mult, op1=OP.add,
                        accum_out=sums[:, j : j + 1],
                    )
                else:
                    nc.vector.reduce_sum(
                        out=sums[:, j : j + 1], in_=xj, axis=mybir.AxisListType.X
                    )
            # stash raw sums into the mu columns (bf16 cast)
            nc.vector.tensor_copy(
                xt[:, :, HW : HW + 2],
                sums[:].unsqueeze(2).to_broadcast([NP, NJ, 2]),
            )
            # lhsT[:, j, q] = m16[q in band(p)] * sums_j  (bf16)
            lhsT = mpool.tile([NP, NJ, NP], BF16, tag="lhsT")
            for j, m in enumerate(LHST_MODE):
                if m == "a":
                    nc.scalar.activation(
                        out=lhsT[:, j, :], in_=m16_sb[:], func=AF.Copy,
                        scale=sums[:, j : j + 1],
                    )
                else:
                    nc.vector.tensor_scalar(
                        out=lhsT[:, j, :], in0=m16_sb[:],
                        scalar1=sums[:, j : j + 1], scalar2=None, op0=OP.mult,
                    )
            state[b] = lhsT

        def phase2(b):
            # s' = HW*s (replicated per 16-band) in cols 0:HW; HW^2*mu in col HW
            xt = xts[b]
            lhsT = state[b]
            ps = spsum.tile([NP, HW + 2], F32)
            for c0 in range(0, HW + 2, MMCHUNK):
                c1 = min(c0 + MMCHUNK, HW + 2)
                for j in range(NJ):
                    nc.tensor.matmul(
                        ps[:, c0:c1], lhsT[:, j, :], xt[:, j, c0:c1],
                        start=(j == 0), stop=(j == NJ - 1),
                    )
            state[b] = ps

        pair_state = {}

        def phase3a(b):
            # per-batch stats: nmu and HW*var accumulated into pair tiles
            ps = state[b]
            k = b % 2
            if k == 0:
                nmu_p = vpool.tile([NP, 2], F32, tag="nmu_p")
                hwvar_p = vpool.tile([NP, 2], F32, tag="hwvar_p")
                pair_state[b // 2] = (nmu_p, hwvar_p, None, None)
            nmu_p, hwvar_p, _, _ = pair_state[b // 2]
            nc.scalar.activation(
                out=nmu_p[:, k : k + 1], in_=ps[:, HW : HW + 1], func=AF.Copy,
                scale=-1.0 / HW,
            )
            sq = gpool.tile([NP, HW], BF16, tag="sq")
            nc.scalar.activation(
                out=sq[:], in_=ps[:, 0:HW], func=AF.Square,
                bias=nmu_p[:, k : k + 1], accum_out=hwvar_p[:, k : k + 1],
            )

        def phase3b(p):
            # pair-batched rsqrt: bit-trick seed + Newton on [NP, 2]
            # (eps dropped: v = HW^3*(var+~0) and var >> eps for this data)
            nmu_p, hwvar_p, _, _ = pair_state[p]
            a_t = vpool.tile([NP, 2], F32, tag="a_t")
            if RSQRT_MODE == "pow":
                # a = w*sqrt(HW) * v^-0.5 in a single DVE op
                nc.vector.tensor_scalar(
                    out=a_t[:], in0=hwvar_p[:], scalar1=-0.5,
                    scalar2=wv_sb[:, 0:1], op0=OP.pow, op1=OP.mult,
                )
            else:
                y_t = vpool.tile([NP, 2], F32, tag="y_t")
                nc.vector.tensor_scalar(
                    out=y_t[:].bitcast(I32), in0=hwvar_p[:].bitcast(I32),
                    scalar1=1, scalar2=-1,
                    op0=OP.logical_shift_right, op1=OP.bitwise_xor,
                )
                nc.vector.tensor_scalar_add(
                    y_t[:].bitcast(I32), y_t[:].bitcast(I32), RSQRT_MAGIC + 1
                )
                # Newton: y <- y * (1.5 - 0.5*v*y^2); last mul folds w*sqrt(HW)
                t1 = vpool.tile([NP, 2], F32, tag="t1")
                u_t = vpool.tile([NP, 2], F32, tag="u_t")
                y = y_t
                for it in range(NR_ITERS):
                    nc.scalar.activation(out=t1[:], in_=y[:], func=AF.Square)
                    nc.vector.scalar_tensor_tensor(
                        out=u_t[:], in0=t1[:], scalar=-0.5, in1=hwvar_p[:],
                        op0=OP.mult, op1=OP.mult,
                    )
                    nc.vector.tensor_scalar_add(u_t[:], u_t[:], 1.5)
                    if it < NR_ITERS - 1:
                        yn = vpool.tile([NP, 2], F32, tag=f"y{it}")
                        nc.vector.tensor_mul(yn[:], y[:], u_t[:])
                        y = yn
                nc.vector.scalar_tensor_tensor(
                    out=a_t[:], in0=y[:], scalar=wv_sb[:, 0:1], in1=u_t[:],
                    op0=OP.mult, op1=OP.mult,
                )
            c_t = vpool.tile([NP, 2], F32, tag="c_t")
            nc.vector.tensor_mul(c_t[:], nmu_p[:], a_t[:])
            nc.vector.tensor_scalar(
                out=c_t[:], in0=c_t[:], scalar1=bv_sb[:, 0:1], scalar2=None,
                op0=OP.add,
            )
            pair_state[p] = (nmu_p, hwvar_p, a_t, c_t)

        def phase3c(b):
            # per-batch gate from the pair's a/c columns
            ps = state[b]
            k = b % 2
            _, _, a_t, c_t = pair_state[b // 2]
            gate = gpool.tile([NP, HW], BF16, tag="gate")
            nc.scalar.activation(
                out=gate[:], in_=ps[:, 0:HW], func=AF.Sigmoid,
                bias=c_t[:, k : k + 1], scale=a_t[:, k : k + 1],
            )
            state[b] = gate

        dma_eng = nc.scalar if OUT_ENGINE == "scalar" else nc.sync

        def pairmul(eng, ot, xt, gate, j0):
            eng.tensor_mul(
                ot[:, j0 : j0 + 2, :], xt[:, j0 : j0 + 2, 0:HW],
                gate[:].unsqueeze(1).to_broadcast([NP, 2, HW]),
            )

        def phase4a(b):
            # first half of the gating multiply + store of j0/j1
            xt = xts[b]
            gate = state[b]
            ot = opool.tile([NP, NJ, HW], BF16)
            state[b] = (gate, ot)
            if MUL_MODE == "4v":
                nc.vector.tensor_mul(
                    ot[:], xt[:, :, 0:HW],
                    gate[:].unsqueeze(1).to_broadcast([NP, NJ, HW]),
                )
                dma_eng.dma_start(out=ys[b], in_=ot[:])
                return
            if MUL_MODE == "vvvv":
                nc.vector.tensor_mul(ot[:, 0, :], xt[:, 0, 0:HW], gate[:])
                nc.vector.tensor_mul(ot[:, 1, :], xt[:, 1, 0:HW], gate[:])
            else:
                pairmul(nc.vector, ot, xt, gate, 0)
            dma_eng.dma_start(out=ys[b, :, 0:2, :], in_=ot[:, 0:2, :])

        def phase4b(b):
            # second half (j2/j3) + store; GpSimd op (if any) emitted first
            xt = xts.pop(b)
            gate, ot = state.pop(b)
            if MUL_MODE == "4v":
                if b + PREF < BLOC:
                    dma_in(b + PREF)
                return
            if MUL_MODE == "2vvg":
                nc.gpsimd.tensor_mul(ot[:, 3, :], xt[:, 3, 0:HW], gate[:])
                nc.vector.tensor_mul(ot[:, 2, :], xt[:, 2, 0:HW], gate[:])
            elif MUL_MODE == "2v2v":
                pairmul(nc.vector, ot, xt, gate, 2)
            elif MUL_MODE == "2v2g":
                pairmul(nc.gpsimd, ot, xt, gate, 2)
            else:
                nc.vector.tensor_mul(ot[:, 2, :], xt[:, 2, 0:HW], gate[:])
                nc.vector.tensor_mul(ot[:, 3, :], xt[:, 3, 0:HW], gate[:])
            dma_eng.dma_start(out=ys[b, :, 2:4, :], in_=ot[:, 2:4, :])
            if b + PREF < BLOC:
                dma_in(b + PREF)

        # software-pipelined emission: each engine's stream sees work in
        # data-readiness order, so in-order engines never head-of-line block.
        # m8 carries the [NP, NP] block-banded 0/1 indicator
        # M16[p, q] = (p//PBAND == q//PBAND); wv (= w*sqrt(HW)) and bv are
        # 16x-replicated [NP, 1]
        m16_sb = consts.tile([NP, NP], BF16)
        nc.sync.dma_start(out=m16_sb[:], in_=m8[:])
        wv_sb = consts.tile([NP, 1], F32)
        nc.sync.dma_start(out=wv_sb[:], in_=wv[:])
        bv_sb = consts.tile([NP, 1], F32)
        nc.sync.dma_start(out=bv_sb[:], in_=bv[:])
        for b in range(min(PREF, BLOC)):
            dma_in(b)
        phase1(0)
        phase2(0)
        phase1(1)
        phase2(1)
        for p in range(BLOC // 2):
            b0, b1 = 2 * p, 2 * p + 1
            phase3a(b0)
            if b0 + 2 < BLOC:
                phase1(b0 + 2)
            phase3a(b1)
            if b0 + 2 < BLOC:
                phase2(b0 + 2)
            phase3b(p)
            phase3c(b0)
            phase4a(b0)
            if b1 + 2 < BLOC:
                phase1(b1 + 2)
            phase4b(b0)
            phase3c(b1)
            phase4a(b1)
            if b1 + 2 < BLOC:
                phase2(b1 + 2)
            phase4b(b1)


def _build_nc():
    nc = bacc.Bacc("TRN2", debug=False)
    xs = nc.dram_tensor("xs", [BLOC, NP, NJ, HW], BF16, kind="ExternalInput")
    m8 = nc.dram_tensor("m8", [NP, NP], BF16, kind="ExternalInput")
    wv = nc.dram_tensor("wv", [NP, 1], F32, kind="ExternalInput")
    bv = nc.dram_tensor("bv", [NP, 1], F32, kind="ExternalInput")
    ys = nc.dram_tensor("ys", [BLOC, NP, NJ, HW], BF16, kind="ExternalOutput")
    with tile.TileContext(nc) as tc:
        _emit(tc, nc, xs, m8, wv, bv, ys)
    nc.compile()
    return nc


def get_nc():
    if "nc" not in _cache:
        _cache["nc"] = _build_nc()
    return _cache["nc"]


def make_in_maps(x, weight, bias):
    x = np.ascontiguousarray(np.asarray(x, dtype=np.float32))
    weight = np.asarray(weight, dtype=np.float32).reshape(G)
    bias = np.asarray(bias, dtype=np.float32).reshape(G)
    # [core, b, p, j, hw] with c = NJ*p + j; downcast to bf16 on host
    xs = x.reshape(NCORES, BLOC, NP, NJ, HW).astype(NPBF16)
    band = np.arange(NP) // PBAND
    m8 = (band[:, None] == band[None, :]).astype(NPBF16)  # [NP, NP] indicator
    wv = np.ascontiguousarray(
        (np.repeat(weight, PBAND) * np.sqrt(float(HW)))[:, None]
    ).astype(np.float32)
    bv = np.ascontiguousarray(np.repeat(bias, PBAND)[:, None])
    return [
        {"xs": np.ascontiguousarray(xs[i]), "m8": m8, "wv": wv, "bv": bv}
        for i in range(NCORES)
    ]


def run(x, weight, bias, trace=False, **spmd_kwargs):
    nc = get_nc()
    in_maps = make_in_maps(x, weight, bias)
    res = run_bass_kernel_spmd(
        nc, in_maps, core_ids=list(range(NCORES)), trace=trace, **spmd_kwargs
    )
    out = np.stack(
        [res.results[i]["ys"].astype(np.float32) for i in range(NCORES)]
    )
    return out.reshape(B, C, H, W), res


def kernel(x, weight, bias, groups=G, **_ignored):
    assert int(groups) == G
    out, _ = run(x, weight, bias, trace=False)
    return out
